# revision 6
# baseline (speedup 1.0000x reference)
"""BayesianLinear Trainium2 kernel, 8-core SPMD (data-parallel over batch).

Per-core computation (4 samples each):
    w_b = weight_mean + noise_b * exp(0.5 * weight_logvar)   (B,O,I)
    out_b = x_b @ w_b^T + bias                               (B,L,O)

v3 design (per core), built from the v1/v2 trace post-mortems:
  - The 16 shared DMA engines stream ~24 GB/s each on the LARGER side of a
    transfer (~170 ns per packet), so the 32 MB of f32 input reads cost a
    fixed ~85 us of engine-pool time; everything else must hide under it.
  - mean is kept in NATURAL layout (bf16 resident) and folded into the
    sampled weight before the PE transpose:  wn = noise*std + mean.
    The mul runs on GpSimd (idle between descriptor emissions) and the add
    on DVE, so the per-chunk latency chain is split across two queues and
    DVE keeps capacity for the PSUM evacuations.
  - x rows are loaded PERMUTED (row l = 4p + m, p = partition) so each
    out-tile partition holds 4 consecutive DRAM rows: the bf16 output
    store becomes one DMA per sample with 8 KB contiguous per-partition
    packets.  Output is stored bf16 (tolerance 2e-2, lands ~4e-3) and
    widened to f32 on the host during the gather.
  - The Tile scheduler orders instructions by its own cost model, so the
    intended pipeline is pinned with tile_wait_until virtual-time ticks:
    stores are pushed behind the last noise load (they'd otherwise steal
    DMA-engine slots from the pacing-critical load stream), and each
    stage group is laddered in data-arrival order.
  - Sample 0 starts with quarter-width (N=256) GEMMs gated on single 1 MB
    chunks; sample 3 ends with quarter GEMMs so only a few us of work
    depends on the final noise bytes.  Middle samples use full-width
    tiles (shared stationary, fewer exposed LDWEIGHTS).
"""
import numpy as np

SAMPLES = 4           # batch samples per core
N_CORES = 8
B, L, I, O = 32, 512, 1024, 1024
KT = I // 128         # 8 k-tiles (contraction)
OT = O // 128         # 8 o-blocks
LT = L // 128         # 4 l-tile groups (interleaved rows 4p+m)

_cache = {}


def _split_multi_waits(nc, mybir):
    """This walrus build allows at most one sync-wait per instruction; move
    extra waits onto preceding single-wait NOPs on the same engine.  Safe
    because kernel semaphores are monotonic between resets, so waiting
    sequentially is equivalent to waiting on the conjunction."""
    for fn in nc.m.functions:
        for bb in fn.blocks:
            insts = bb.instructions
            changed = False
            new_list = []
            for inst in insts:
                si = inst.sync_info
                if si is not None and si.on_wait and len(si.on_wait) > 1:
                    waits = list(si.on_wait)
                    for j, w in enumerate(waits[:-1]):
                        nop = mybir.InstNoOp(name=f"{inst.name}-w{j}", ins=[], outs=[])
                        nop.engine = inst.engine
                        nop.sync_info = mybir.SyncInfo(on_wait=[w], on_update=[])
                        new_list.append(nop)
                    inst.sync_info = mybir.SyncInfo(
                        on_wait=[waits[-1]], on_update=list(si.on_update or []))
                    changed = True
                new_list.append(inst)
            if changed:
                bb.instructions = new_list


def build_nc(use_f32r=True):
    from contextlib import ExitStack
    from concourse import bass, mybir, tile, masks

    F32 = mybir.dt.float32
    BF16 = mybir.dt.bfloat16
    Exp = mybir.ActivationFunctionType.Exp
    Copy = mybir.ActivationFunctionType.Copy
    mult = mybir.AluOpType.mult
    add = mybir.AluOpType.add

    nc = bass.Bass()
    x_d = nc.declare_dram_parameter("x", [SAMPLES, L, I], F32, isOutput=False)
    nz_d = nc.declare_dram_parameter("noise", [SAMPLES, O, I], F32, isOutput=False)
    wm_d = nc.declare_dram_parameter("weight_mean", [O, I], F32, isOutput=False)
    wl_d = nc.declare_dram_parameter("weight_logvar", [O, I], F32, isOutput=False)
    b_d = nc.declare_dram_parameter("bias", [O], F32, isOutput=False)
    out_d = nc.declare_dram_parameter("out", [SAMPLES, L, O], BF16, isOutput=True)

    with tile.TileContext(nc) as tc, ExitStack() as ctx:
        resident = ctx.enter_context(tc.tile_pool(name="resident", bufs=1))
        lv_pool = ctx.enter_context(tc.tile_pool(name="lv", bufs=2))
        nz_pool = ctx.enter_context(tc.tile_pool(name="nz", bufs=2))
        wn_pool = ctx.enter_context(tc.tile_pool(name="wn", bufs=3))
        xn_pool = ctx.enter_context(tc.tile_pool(name="xn", bufs=2))
        xT_pool = ctx.enter_context(tc.tile_pool(name="xT", bufs=2))
        wT_pool = ctx.enter_context(tc.tile_pool(name="wT", bufs=2))
        out_pool = ctx.enter_context(tc.tile_pool(name="outp", bufs=4))
        psum_mm = ctx.enter_context(tc.tile_pool(name="psum_mm", bufs=2, space="PSUM"))
        psum_nt = ctx.enter_context(tc.tile_pool(name="psum_nt", bufs=2, space="PSUM"))
        psum_xt = ctx.enter_context(tc.tile_pool(name="psum_xt", bufs=2, space="PSUM"))

        # ---------------- residents ----------------
        std_b = resident.tile([128, OT, I], BF16, tag="std")     # exp(.5 lv), natural
        mean_b = resident.tile([128, OT, I], BF16, tag="mean")   # mean, natural
        ident_b = resident.tile([128, 128], BF16, tag="ident_b")
        ones_b = resident.tile([1, 128], BF16, tag="ones_b")
        bias_f = resident.tile([1, O], F32, tag="bias_f")
        bias_b = resident.tile([1, O], BF16, tag="bias_b")
        bias_blk = resident.tile([128, O], F32, tag="bias_blk")  # bias bcast to rows

        lv_tiles, nz_tiles, x_tiles, wn_tiles = {}, {}, {}, {}

        US = 0.001  # one microsecond of scheduler virtual time, in ms

        def at(t_us):
            return tc.tile_wait_until(t_us * US)

        # ---------------- DMA emitters ----------------
        def emit_lv_load(j):
            lv_tiles[j] = lv_pool.tile([128, 2, I], F32, tag="lv", name=f"lv{j}")
            nc.sync.dma_start(
                lv_tiles[j][:],
                wl_d[256 * j:256 * (j + 1), :].rearrange("(q p) i -> p q i", p=128))

        def emit_mn_load(j):
            # cast straight into the resident natural-layout slab
            nc.gpsimd.dma_start(
                mean_b[:, 2 * j:2 * (j + 1), :],
                wm_d[256 * j:256 * (j + 1), :].rearrange("(q p) i -> p q i", p=128))

        def emit_nz_load(b, clo, chi):
            if b not in nz_tiles:
                nz_tiles[b] = nz_pool.tile([128, OT, I], BF16, tag="nz",
                                           name=f"nz{b}")
            nc.gpsimd.dma_start(
                nz_tiles[b][:, 2 * clo:2 * chi, :],
                nz_d[b, 256 * clo:256 * chi, :].rearrange("(q p) i -> p q i", p=128))

        def emit_x_load(b, mlo, mhi):
            # permuted row mapping: row l = 4p + m -> 16 KB contiguous reads
            # per partition and DRAM-contiguous store packets later
            if b not in x_tiles:
                x_tiles[b] = xn_pool.tile([128, LT, I], BF16, tag="xn",
                                          name=f"xn{b}")
            nc.gpsimd.dma_start(
                x_tiles[b][:, mlo:mhi, :],
                x_d[b].rearrange("(p m) i -> p m i", p=128)[:, mlo:mhi, :])

        # ---------------- compute emitters ----------------
        def emit_exp(j):
            nc.scalar.activation(std_b[:, 2 * j:2 * (j + 1), :], lv_tiles.pop(j)[:],
                                 Exp, bias=0.0, scale=0.5)

        def emit_mul(b, c):
            """wn = noise_chunk * std (GpSimd, between descriptor emissions)."""
            nz = nz_tiles[b] if c < 3 else nz_tiles.pop(b)
            wn = wn_pool.tile([128, 2, I], BF16, tag="wn", name=f"wn{b}_{c}")
            nc.gpsimd.tensor_tensor(wn[:], nz[:, 2 * c:2 * (c + 1), :],
                                    std_b[:, 2 * c:2 * (c + 1), :], mult)
            wn_tiles[(b, c)] = wn

        def emit_add(b, c):
            """wn += mean (DVE)."""
            wn = wn_tiles[(b, c)]
            nc.vector.tensor_tensor(wn[:], wn[:],
                                    mean_b[:, 2 * c:2 * (c + 1), :], add)

        def emit_w_group(b, c, q, wT):
            """transpose wn chunk c half q -> wT o-block 2c+q (PE + ACT evac)."""
            ob = 2 * c + q
            wn = wn_tiles[(b, c)] if q == 0 else wn_tiles.pop((b, c))
            pt = psum_nt.tile([128, KT, 128], BF16, tag="pnt")
            for k in range(KT):
                nc.tensor.matmul(pt[:, k, :], wn[:, q, 128 * k:128 * (k + 1)],
                                 ident_b[:], is_transpose=True, start=True, stop=True)
            nc.scalar.activation(wT[:, :, 128 * ob:128 * (ob + 1)], pt[:], Copy)

        def emit_xT_group(b, m, xT):
            xn = x_tiles.pop(b) if m == LT - 1 else x_tiles[b]
            pt = psum_xt.tile([128, KT, 128], BF16, tag="pxt")
            for k in range(KT):
                nc.tensor.matmul(pt[:, k, :], xn[:, m, 128 * k:128 * (k + 1)],
                                 ident_b[:], is_transpose=True, start=True, stop=True)
            nc.scalar.activation(xT[:, :, 128 * m:128 * (m + 1)], pt[:], Copy)

        def emit_gemm_full(m, wT, xT, ot):
            """all 1024 out-cols of tile m: shared stationary, 2 PSUM banks."""
            pm = psum_mm.tile([128, 2, 512], F32, tag="pmm")
            for k in range(KT):
                for n in range(2):
                    nc.tensor.matmul(pm[:, n, :], xT[:, k, 128 * m:128 * (m + 1)],
                                     wT[:, k, 512 * n:512 * (n + 1)],
                                     start=(k == 0), stop=(k == KT - 1))
            nc.vector.tensor_tensor(ot[:, m, :], pm[:].rearrange("p a b -> p (a b)"),
                                    bias_blk[:], add)

        def emit_gemm_half(m, n, wT, xT, ot):
            pm = psum_mm.tile([128, 2, 512], F32, tag="pmm")
            for k in range(KT):
                nc.tensor.matmul(pm[:, 0, :], xT[:, k, 128 * m:128 * (m + 1)],
                                 wT[:, k, 512 * n:512 * (n + 1)],
                                 start=(k == 0), stop=(k == KT - 1))
            nc.vector.tensor_tensor(ot[:, m, 512 * n:512 * (n + 1)], pm[:, 0, :],
                                    bias_blk[:, 512 * n:512 * (n + 1)], add)

        def emit_gemm_quarter(m, qo, wT, xT, ot):
            pm = psum_mm.tile([128, 2, 512], F32, tag="pmm")
            for k in range(KT):
                nc.tensor.matmul(pm[:, 0, 0:256], xT[:, k, 128 * m:128 * (m + 1)],
                                 wT[:, k, 256 * qo:256 * (qo + 1)],
                                 start=(k == 0), stop=(k == KT - 1))
            nc.vector.tensor_tensor(ot[:, m, 256 * qo:256 * (qo + 1)],
                                    pm[:, 0, 0:256],
                                    bias_blk[:, 256 * qo:256 * (qo + 1)], add)

        def emit_store(b, ot, mlo=0, mhi=LT):
            # row l = 4p + m: per-partition (m, o) block is DRAM-contiguous
            nc.scalar.dma_start(
                out_d[b].rearrange("(p m) o -> p m o", p=128)[:, mlo:mhi, :],
                ot[:, mlo:mhi, :])

        # ---------------- DMA stream (gpsimd order = stream order) --------
        with at(0):
            emit_x_load(0, 0, 1)
            emit_lv_load(0)
            nc.sync.dma_start(bias_f[:], b_d[:].rearrange("(a o) -> a o", a=1))
            masks.make_identity(nc, ident_b[:])
            emit_nz_load(0, 0, 1)
            emit_mn_load(0)
            nc.vector.memset(ones_b[:], 1.0)
        with at(2):
            emit_x_load(0, 1, 2)
            emit_nz_load(0, 1, 2)
            emit_mn_load(1)
            emit_lv_load(1)
        with at(4):
            emit_nz_load(0, 2, 3)
            emit_mn_load(2)
        # setup compute
        with at(9):
            nc.vector.tensor_copy(bias_b[:], bias_f[:])
            for n in range(2):
                pb = psum_mm.tile([128, 2, 512], F32, tag="pmm", name=f"pb{n}")
                nc.tensor.matmul(pb[:, 0, :], ones_b[:],
                                 bias_b[:, 512 * n:512 * (n + 1)],
                                 start=True, stop=True)
                nc.scalar.activation(bias_blk[:, 512 * n:512 * (n + 1)],
                                     pb[:, 0, :], Copy)
            pw = psum_mm.tile([128, 2, 512], F32, tag="pmm", name="pw")
            for _ in range(4):
                nc.tensor.matmul(pw[:, 0, 0:128], ident_b[:], ident_b[:],
                                 start=True, stop=True)

        # ---------------- sample 0: fine-grained rampup ----------------
        def fresh(b):
            wT = wT_pool.tile([128, KT, O], BF16, tag="wT", name=f"wT{b}")
            xT = xT_pool.tile([128, KT, L], BF16, tag="xT", name=f"xT{b}")
            ot = out_pool.tile([128, LT, O], BF16, tag="out", name=f"ot{b}")
            return wT, xT, ot

        wT0, xT0, ot0 = fresh(0)
        with at(12):
            emit_xT_group(0, 0, xT0)
            emit_exp(0)
        with at(13):
            emit_exp(1)
            emit_lv_load(2)
        with at(15):
            emit_mul(0, 0)          # gpsimd, after the early emissions
            emit_add(0, 0)
            emit_x_load(0, 2, 3)    # stream: x0m2 after mn2
            emit_nz_load(0, 3, 4)
            emit_mn_load(3)
        with at(16):
            emit_xT_group(0, 1, xT0)
            emit_w_group(0, 0, 0, wT0)
            emit_w_group(0, 0, 1, wT0)
        with at(17):
            emit_gemm_quarter(0, 0, wT0, xT0, ot0)
            emit_exp(2)
        with at(18):
            emit_mul(0, 1)
            emit_add(0, 1)
            emit_x_load(0, 3, 4)
        with at(19):
            emit_w_group(0, 1, 0, wT0)
            emit_w_group(0, 1, 1, wT0)
            emit_lv_load(3)
        with at(20):
            emit_gemm_quarter(0, 1, wT0, xT0, ot0)
            emit_gemm_quarter(1, 0, wT0, xT0, ot0)
            emit_gemm_quarter(1, 1, wT0, xT0, ot0)
            emit_exp(3)
        with at(21):
            emit_mul(0, 2)
            emit_add(0, 2)
            emit_nz_load(1, 0, 2)   # s1 stream begins
        with at(22):
            emit_w_group(0, 2, 0, wT0)
            emit_w_group(0, 2, 1, wT0)
            emit_xT_group(0, 2, xT0)
        with at(23):
            emit_gemm_half(2, 0, wT0, xT0, ot0)
        with at(24):
            emit_mul(0, 3)
            emit_add(0, 3)
            emit_x_load(1, 0, 4)
        with at(25):
            emit_w_group(0, 3, 0, wT0)
            emit_w_group(0, 3, 1, wT0)
            emit_xT_group(0, 3, xT0)
        with at(26):
            emit_gemm_half(3, 0, wT0, xT0, ot0)
            emit_nz_load(1, 2, 4)
        with at(28):
            emit_gemm_half(0, 1, wT0, xT0, ot0)
            emit_gemm_half(1, 1, wT0, xT0, ot0)
        with at(30):
            emit_gemm_half(2, 1, wT0, xT0, ot0)
            emit_gemm_half(3, 1, wT0, xT0, ot0)

        # ---------------- samples 1..3 ----------------
        # per-sample stream slot: c01 ~ t, x ~ t+5, c23 ~ t+10 (us-ish)
        base = {1: 40, 2: 57, 3: 74}
        ots = {0: ot0}
        for b in (1, 2, 3):
            wT, xT, ot = fresh(b)
            ots[b] = ot
            t = base[b]
            with at(t):
                emit_mul(b, 0)
                emit_add(b, 0)
                if b < 3:
                    emit_nz_load(b + 1, 0, 2)
            with at(t + 1):
                emit_w_group(b, 0, 0, wT)
                emit_w_group(b, 0, 1, wT)
            with at(t + 3):
                emit_mul(b, 1)
                emit_add(b, 1)
            with at(t + 4):
                emit_w_group(b, 1, 0, wT)
                emit_w_group(b, 1, 1, wT)
            with at(t + 6):
                if b < 3:
                    emit_x_load(b + 1, 0, 4)
                for m in range(LT):
                    emit_xT_group(b, m, xT)
            if b == 2:
                # full-width tiles: wait for c23 (PE lag covers it)
                with at(t + 9):
                    emit_mul(b, 2)
                    emit_add(b, 2)
                    emit_nz_load(b + 1, 2, 3)
                    emit_nz_load(b + 1, 3, 4)
                with at(t + 10):
                    emit_w_group(b, 2, 0, wT)
                    emit_w_group(b, 2, 1, wT)
                with at(t + 11):
                    emit_mul(b, 3)
                    emit_add(b, 3)
                with at(t + 12):
                    emit_w_group(b, 3, 0, wT)
                    emit_w_group(b, 3, 1, wT)
                with at(t + 13):
                    for m in range(LT):
                        emit_gemm_full(m, wT, xT, ot)
            else:
                with at(t + 8):
                    for m in range(LT):
                        emit_gemm_half(m, 0, wT, xT, ot)
                with at(t + 10):
                    emit_mul(b, 2)
                    emit_add(b, 2)
                    if b < 3:
                        emit_nz_load(b + 1, 2, 4)
                with at(t + 11):
                    emit_w_group(b, 2, 0, wT)
                    emit_w_group(b, 2, 1, wT)
                with at(t + 12):
                    emit_mul(b, 3)
                    emit_add(b, 3)
                with at(t + 13):
                    emit_w_group(b, 3, 0, wT)
                    emit_w_group(b, 3, 1, wT)
                if b < 3:
                    with at(t + 14):
                        for m in range(LT):
                            emit_gemm_half(m, 1, wT, xT, ot)
                else:
                    # sample 3 tail: quarter GEMMs so only chunk-3-dependent
                    # work sits behind the final noise bytes
                    with at(t + 12):
                        for m in range(LT):
                            emit_gemm_quarter(m, 2, wT, xT, ot)
                    with at(t + 15):
                        emit_gemm_quarter(0, 3, wT, xT, ot)
                        emit_gemm_quarter(1, 3, wT, xT, ot)
                    with at(t + 17):
                        emit_store(3, ot, 0, 2)
                        emit_gemm_quarter(2, 3, wT, xT, ot)
                        emit_gemm_quarter(3, 3, wT, xT, ot)
                    with at(t + 19):
                        emit_store(3, ot, 2, 4)

        # deferred stores: behind the last noise load in the stream
        with at(88):
            emit_store(0, ots[0])
        with at(90):
            emit_store(1, ots[1])
        with at(92):
            emit_store(2, ots[2])

    _split_multi_waits(nc, mybir)
    return nc


def _get_nc(use_f32r=True):
    key = ("nc", use_f32r)
    if key not in _cache:
        _cache[key] = build_nc(use_f32r)
    return _cache[key]


def kernel(x, weight_mean, weight_logvar, bias, noise):
    from concourse import bass_utils

    x = np.ascontiguousarray(x, dtype=np.float32)
    noise = np.ascontiguousarray(noise, dtype=np.float32)
    weight_mean = np.ascontiguousarray(weight_mean, dtype=np.float32)
    weight_logvar = np.ascontiguousarray(weight_logvar, dtype=np.float32)
    bias = np.ascontiguousarray(bias, dtype=np.float32)

    nc = _get_nc()
    in_maps = []
    for c in range(N_CORES):
        sl = slice(SAMPLES * c, SAMPLES * (c + 1))
        in_maps.append({
            "x": x[sl], "noise": noise[sl],
            "weight_mean": weight_mean, "weight_logvar": weight_logvar,
            "bias": bias,
        })
    res = bass_utils.run_bass_kernel_spmd(nc, in_maps, list(range(N_CORES)))
    out = np.concatenate([np.asarray(res.results[c]["out"]).astype(np.float32)
                          for c in range(N_CORES)], axis=0)
    return out


# revision 9
# speedup vs baseline: 1.1450x; 1.1450x over previous
"""BayesianLinear Trainium2 kernel, 8-core SPMD (data-parallel over batch).

Per-core computation (4 samples each):
    w_b = weight_mean + noise_b * exp(0.5 * weight_logvar)   (B,O,I)
    out_b = x_b @ w_b^T + bias                               (B,L,O)

v4 design (per core), from the v1/v2/v3 trace post-mortems:
  - The 16 shared DMA engines stream ~24 GB/s each on the LARGER side of a
    transfer (~170 ns per packet), so the 32 MB of f32 input reads cost a
    fixed ~85 us of engine-pool time; everything else must hide under it.
  - Engine assignment by measured elementwise rates (DVE ~1.6 el/part/ns
    bf16, scalar ACT ~0.9, gpsimd ~0.5): DVE does the noise*std mul, the
    fused PSUM-evac + mean^T add, and the GEMM evac + bias add; scalar
    does exp, the mean^T/x^T evacs and store descriptors; gpsimd only
    pumps SWDGE descriptors.  (v3 taught us gpsimd tensor ops are 4 us.)
  - x rows are loaded PERMUTED (row l = 4p + m, p = partition) so each
    out-tile partition holds 4 consecutive DRAM rows: the bf16 output
    store is one DMA per sample with 8 KB contiguous per-partition
    packets.  Output is stored bf16 (tolerance 2e-2, lands ~4e-3) and
    widened to f32 on the host during the gather.
  - The Tile scheduler orders instructions by its own cost model, so the
    intended pipeline is pinned with tile_wait_until virtual-time ticks:
    stores are pushed behind the last noise load (they'd otherwise steal
    DMA-engine slots from the pacing-critical load stream) and drain
    during sample 3's compute tail; every stage group is laddered in
    data-arrival order so the in-order queues never head-of-line block.
  - Sample 0 ramps with quarter-width (N=256) GEMMs gated on single 1 MB
    chunks; sample 3 ends with quarter GEMMs so only a few us of work
    depends on the final noise bytes.  Middle samples use full-width
    tiles (shared stationary, fewer exposed LDWEIGHTS).
"""
import numpy as np

SAMPLES = 4           # batch samples per core
N_CORES = 8
B, L, I, O = 32, 512, 1024, 1024
KT = I // 128         # 8 k-tiles (contraction)
OT = O // 128         # 8 o-blocks
LT = L // 128         # 4 l-tile groups (interleaved rows 4p+m)

_cache = {}


def _split_multi_waits(nc, mybir):
    """This walrus build allows at most one sync-wait per instruction; move
    extra waits onto preceding single-wait NOPs on the same engine.  Safe
    because kernel semaphores are monotonic between resets, so waiting
    sequentially is equivalent to waiting on the conjunction."""
    for fn in nc.m.functions:
        for bb in fn.blocks:
            insts = bb.instructions
            changed = False
            new_list = []
            for inst in insts:
                si = inst.sync_info
                if si is not None and si.on_wait and len(si.on_wait) > 1:
                    waits = list(si.on_wait)
                    for j, w in enumerate(waits[:-1]):
                        nop = mybir.InstNoOp(name=f"{inst.name}-w{j}", ins=[], outs=[])
                        nop.engine = inst.engine
                        nop.sync_info = mybir.SyncInfo(on_wait=[w], on_update=[])
                        new_list.append(nop)
                    inst.sync_info = mybir.SyncInfo(
                        on_wait=[waits[-1]], on_update=list(si.on_update or []))
                    changed = True
                new_list.append(inst)
            if changed:
                bb.instructions = new_list


def build_nc(use_f32r=True):
    from contextlib import ExitStack
    from concourse import bass, mybir, tile, masks

    F32 = mybir.dt.float32
    BF16 = mybir.dt.bfloat16
    Exp = mybir.ActivationFunctionType.Exp
    Copy = mybir.ActivationFunctionType.Copy
    mult = mybir.AluOpType.mult
    add = mybir.AluOpType.add

    nc = bass.Bass()
    x_d = nc.declare_dram_parameter("x", [SAMPLES, L, I], F32, isOutput=False)
    nz_d = nc.declare_dram_parameter("noise", [SAMPLES, O, I], F32, isOutput=False)
    wm_d = nc.declare_dram_parameter("weight_mean", [O, I], F32, isOutput=False)
    wl_d = nc.declare_dram_parameter("weight_logvar", [O, I], F32, isOutput=False)
    b_d = nc.declare_dram_parameter("bias", [O], F32, isOutput=False)
    out_d = nc.declare_dram_parameter("out", [SAMPLES, L, O], BF16, isOutput=True)

    with tile.TileContext(nc) as tc, ExitStack() as ctx:
        resident = ctx.enter_context(tc.tile_pool(name="resident", bufs=1))
        lv_pool = ctx.enter_context(tc.tile_pool(name="lv", bufs=1))
        mn_pool = ctx.enter_context(tc.tile_pool(name="mn", bufs=3))
        nz_pool = ctx.enter_context(tc.tile_pool(name="nz", bufs=2))
        sc_pool = ctx.enter_context(tc.tile_pool(name="sc", bufs=3))
        xn_pool = ctx.enter_context(tc.tile_pool(name="xn", bufs=2))
        xT_pool = ctx.enter_context(tc.tile_pool(name="xT", bufs=2))
        wT_pool = ctx.enter_context(tc.tile_pool(name="wT", bufs=2))
        out_pool = ctx.enter_context(tc.tile_pool(name="outp", bufs=4))
        psum_mm = ctx.enter_context(tc.tile_pool(name="psum_mm", bufs=2, space="PSUM"))
        psum_nt = ctx.enter_context(tc.tile_pool(name="psum_nt", bufs=2, space="PSUM"))
        psum_xt = ctx.enter_context(tc.tile_pool(name="psum_xt", bufs=2, space="PSUM"))

        # ---------------- residents ----------------
        std_b = resident.tile([128, OT, I], BF16, tag="std")     # exp(.5 lv), natural
        meanT = resident.tile([128, KT, O], BF16, tag="meanT")   # mean^T
        ident_b = resident.tile([128, 128], BF16, tag="ident_b")
        ones_b = resident.tile([1, 128], BF16, tag="ones_b")
        bias_f = resident.tile([1, O], F32, tag="bias_f")
        bias_b = resident.tile([1, O], BF16, tag="bias_b")
        bias_blk = resident.tile([128, O], F32, tag="bias_blk")  # bias bcast to rows

        lv_tiles, mn_tiles, nz_tiles, x_tiles, sc_tiles = {}, {}, {}, {}, {}

        US = 0.001  # one microsecond of scheduler virtual time, in ms

        def at(t_us):
            return tc.tile_wait_until(t_us * US)

        # ---------------- DMA emitters ----------------
        def emit_lv_load(j):
            lv_tiles[j] = lv_pool.tile([128, 2, I], F32, tag="lv", name=f"lv{j}")
            nc.sync.dma_start(
                lv_tiles[j][:],
                wl_d[256 * j:256 * (j + 1), :].rearrange("(q p) i -> p q i", p=128))

        def emit_mn_load(j):
            mn_tiles[j] = mn_pool.tile([128, 2, I], BF16, tag="mn", name=f"mn{j}")
            nc.gpsimd.dma_start(
                mn_tiles[j][:],
                wm_d[256 * j:256 * (j + 1), :].rearrange("(q p) i -> p q i", p=128))

        def emit_nz_load(b, clo, chi):
            if b not in nz_tiles:
                nz_tiles[b] = nz_pool.tile([128, OT, I], BF16, tag="nz",
                                           name=f"nz{b}")
            nc.gpsimd.dma_start(
                nz_tiles[b][:, 2 * clo:2 * chi, :],
                nz_d[b, 256 * clo:256 * chi, :].rearrange("(q p) i -> p q i", p=128))

        def emit_x_load(b, mlo, mhi):
            # permuted row mapping: row l = 4p + m -> 16 KB contiguous reads
            # per partition and DRAM-contiguous store packets later
            if b not in x_tiles:
                x_tiles[b] = xn_pool.tile([128, LT, I], BF16, tag="xn",
                                          name=f"xn{b}")
            nc.gpsimd.dma_start(
                x_tiles[b][:, mlo:mhi, :],
                x_d[b].rearrange("(p m) i -> p m i", p=128)[:, mlo:mhi, :])

        # ---------------- compute emitters ----------------
        def emit_exp(j):
            nc.scalar.activation(std_b[:, 2 * j:2 * (j + 1), :], lv_tiles.pop(j)[:],
                                 Exp, bias=0.0, scale=0.5)

        def emit_mean_group(j, q):
            """transpose mean slab j half q -> meanT o-block 2j+q (PE + ACT)."""
            ob = 2 * j + q
            mn = mn_tiles[j] if q == 0 else mn_tiles.pop(j)
            pt = psum_nt.tile([128, KT, 128], BF16, tag="pnt")
            for k in range(KT):
                nc.tensor.matmul(pt[:, k, :], mn[:, q, 128 * k:128 * (k + 1)],
                                 ident_b[:], is_transpose=True, start=True, stop=True)
            nc.scalar.activation(meanT[:, :, 128 * ob:128 * (ob + 1)], pt[:], Copy)

        def emit_mul(b, c):
            """sc = noise_chunk * std (DVE, bf16)."""
            nz = nz_tiles[b] if c < 3 else nz_tiles.pop(b)
            sc = sc_pool.tile([128, 2, I], BF16, tag="sc", name=f"sc{b}_{c}")
            nc.vector.tensor_tensor(sc[:], nz[:, 2 * c:2 * (c + 1), :],
                                    std_b[:, 2 * c:2 * (c + 1), :], mult)
            sc_tiles[(b, c)] = sc

        def emit_w_group(b, c, q, wT):
            """transpose sc chunk c half q, add mean^T -> wT o-block 2c+q."""
            ob = 2 * c + q
            sc = sc_tiles[(b, c)] if q == 0 else sc_tiles.pop((b, c))
            pt = psum_nt.tile([128, KT, 128], BF16, tag="pnt")
            for k in range(KT):
                nc.tensor.matmul(pt[:, k, :], sc[:, q, 128 * k:128 * (k + 1)],
                                 ident_b[:], is_transpose=True, start=True, stop=True)
            nc.vector.tensor_tensor(wT[:, :, 128 * ob:128 * (ob + 1)], pt[:],
                                    meanT[:, :, 128 * ob:128 * (ob + 1)], add)

        def emit_xT_group(b, m, xT):
            xn = x_tiles.pop(b) if m == LT - 1 else x_tiles[b]
            pt = psum_xt.tile([128, KT, 128], BF16, tag="pxt")
            for k in range(KT):
                nc.tensor.matmul(pt[:, k, :], xn[:, m, 128 * k:128 * (k + 1)],
                                 ident_b[:], is_transpose=True, start=True, stop=True)
            nc.scalar.activation(xT[:, :, 128 * m:128 * (m + 1)], pt[:], Copy)

        def emit_gemm_full(m, wT, xT, ot):
            """all 1024 out-cols of tile m: shared stationary, 2 PSUM banks."""
            pm = psum_mm.tile([128, 2, 512], F32, tag="pmm")
            for k in range(KT):
                for n in range(2):
                    nc.tensor.matmul(pm[:, n, :], xT[:, k, 128 * m:128 * (m + 1)],
                                     wT[:, k, 512 * n:512 * (n + 1)],
                                     start=(k == 0), stop=(k == KT - 1))
            nc.vector.tensor_tensor(ot[:, m, :], pm[:].rearrange("p a b -> p (a b)"),
                                    bias_blk[:], add)

        def emit_gemm_half(m, n, wT, xT, ot):
            pm = psum_mm.tile([128, 2, 512], F32, tag="pmm")
            for k in range(KT):
                nc.tensor.matmul(pm[:, 0, :], xT[:, k, 128 * m:128 * (m + 1)],
                                 wT[:, k, 512 * n:512 * (n + 1)],
                                 start=(k == 0), stop=(k == KT - 1))
            nc.vector.tensor_tensor(ot[:, m, 512 * n:512 * (n + 1)], pm[:, 0, :],
                                    bias_blk[:, 512 * n:512 * (n + 1)], add)

        def emit_gemm_quarter(m, qo, wT, xT, ot):
            pm = psum_mm.tile([128, 2, 512], F32, tag="pmm")
            for k in range(KT):
                nc.tensor.matmul(pm[:, 0, 0:256], xT[:, k, 128 * m:128 * (m + 1)],
                                 wT[:, k, 256 * qo:256 * (qo + 1)],
                                 start=(k == 0), stop=(k == KT - 1))
            nc.vector.tensor_tensor(ot[:, m, 256 * qo:256 * (qo + 1)],
                                    pm[:, 0, 0:256],
                                    bias_blk[:, 256 * qo:256 * (qo + 1)], add)

        def emit_store(b, ot, mlo=0, mhi=LT):
            # row l = 4p + m: per-partition (m, o) block is DRAM-contiguous
            nc.scalar.dma_start(
                out_d[b].rearrange("(p m) o -> p m o", p=128)[:, mlo:mhi, :],
                ot[:, mlo:mhi, :])

        # ---------------- DMA stream (gpsimd order = stream order) --------
        with at(0):
            emit_x_load(0, 0, 1)
            emit_lv_load(0)
            nc.sync.dma_start(bias_f[:], b_d[:].rearrange("(a o) -> a o", a=1))
            masks.make_identity(nc, ident_b[:])
            emit_nz_load(0, 0, 1)
            emit_mn_load(0)
            nc.vector.memset(ones_b[:], 1.0)
        with at(2):
            emit_x_load(0, 1, 2)
            emit_nz_load(0, 1, 2)
            emit_mn_load(1)
            emit_lv_load(1)
        with at(4):
            emit_nz_load(0, 2, 3)
            emit_mn_load(2)
        with at(6):
            emit_x_load(0, 2, 3)
            emit_nz_load(0, 3, 4)
        with at(8):
            emit_x_load(0, 3, 4)
            emit_mn_load(3)
        # setup compute
        with at(9):
            nc.vector.tensor_copy(bias_b[:], bias_f[:])
            for n in range(2):
                pb = psum_mm.tile([128, 2, 512], F32, tag="pmm", name=f"pb{n}")
                nc.tensor.matmul(pb[:, 0, :], ones_b[:],
                                 bias_b[:, 512 * n:512 * (n + 1)],
                                 start=True, stop=True)
                nc.scalar.activation(bias_blk[:, 512 * n:512 * (n + 1)],
                                     pb[:, 0, :], Copy)
            pw = psum_mm.tile([128, 2, 512], F32, tag="pmm", name="pw")
            for _ in range(4):
                nc.tensor.matmul(pw[:, 0, 0:128], ident_b[:], ident_b[:],
                                 start=True, stop=True)

        # ---------------- sample 0: fine-grained rampup ----------------
        def fresh(b):
            wT = wT_pool.tile([128, KT, O], BF16, tag="wT", name=f"wT{b}")
            xT = xT_pool.tile([128, KT, L], BF16, tag="xT", name=f"xT{b}")
            ot = out_pool.tile([128, LT, O], BF16, tag="out", name=f"ot{b}")
            return wT, xT, ot

        wT0, xT0, ot0 = fresh(0)
        ots = {0: ot0}
        with at(11):
            emit_exp(0)
        with at(12):
            emit_xT_group(0, 0, xT0)
        with at(13):
            emit_mean_group(0, 0)
            emit_mean_group(0, 1)
        with at(14):
            emit_mul(0, 0)
            emit_exp(1)
            emit_lv_load(2)
        with at(15):
            emit_w_group(0, 0, 0, wT0)
            emit_w_group(0, 0, 1, wT0)
            emit_xT_group(0, 1, xT0)
        with at(16):
            emit_gemm_quarter(0, 0, wT0, xT0, ot0)
            emit_mean_group(1, 0)
            emit_mean_group(1, 1)
        with at(17):
            emit_mul(0, 1)
            emit_exp(2)
            emit_lv_load(3)
        with at(18):
            emit_w_group(0, 1, 0, wT0)
            emit_w_group(0, 1, 1, wT0)
        with at(19):
            emit_gemm_quarter(0, 1, wT0, xT0, ot0)
            emit_gemm_quarter(1, 0, wT0, xT0, ot0)
            emit_gemm_quarter(1, 1, wT0, xT0, ot0)
            emit_mean_group(2, 0)
            emit_mean_group(2, 1)
        with at(20):
            emit_mul(0, 2)
            emit_exp(3)
        with at(21):
            emit_w_group(0, 2, 0, wT0)
            emit_w_group(0, 2, 1, wT0)
            emit_xT_group(0, 2, xT0)
        with at(22):
            emit_gemm_half(2, 0, wT0, xT0, ot0)
            emit_mean_group(3, 0)
            emit_mean_group(3, 1)
        with at(23):
            emit_mul(0, 3)
            emit_nz_load(1, 0, 2)
        with at(24):
            emit_w_group(0, 3, 0, wT0)
            emit_w_group(0, 3, 1, wT0)
            emit_xT_group(0, 3, xT0)
        with at(25):
            emit_gemm_half(3, 0, wT0, xT0, ot0)
            emit_x_load(1, 0, 4)
        with at(27):
            emit_gemm_half(0, 1, wT0, xT0, ot0)
            emit_gemm_half(1, 1, wT0, xT0, ot0)
            emit_nz_load(1, 2, 4)
        with at(29):
            emit_gemm_half(2, 1, wT0, xT0, ot0)
            emit_gemm_half(3, 1, wT0, xT0, ot0)

        # ---------------- samples 1..3 ----------------
        # per-sample stream slot: c01 ~ t, x ~ t+5, c23 ~ t+10 (us-ish)
        base = {1: 40, 2: 57, 3: 74}
        for b in (1, 2, 3):
            wT, xT, ot = fresh(b)
            ots[b] = ot
            t = base[b]
            with at(t):
                emit_mul(b, 0)
                if b < 3:
                    emit_nz_load(b + 1, 0, 2)
            with at(t + 1):
                emit_w_group(b, 0, 0, wT)
                emit_w_group(b, 0, 1, wT)
            with at(t + 3):
                emit_mul(b, 1)
            with at(t + 4):
                emit_w_group(b, 1, 0, wT)
                emit_w_group(b, 1, 1, wT)
            with at(t + 6):
                if b < 3:
                    emit_x_load(b + 1, 0, 4)
                for m in range(LT):
                    emit_xT_group(b, m, xT)
            if b == 2:
                # full-width tiles: wait for c23 (PE lag covers it)
                with at(t + 9):
                    emit_mul(b, 2)
                    emit_nz_load(b + 1, 2, 3)
                    emit_nz_load(b + 1, 3, 4)
                with at(t + 10):
                    emit_w_group(b, 2, 0, wT)
                    emit_w_group(b, 2, 1, wT)
                with at(t + 11):
                    emit_mul(b, 3)
                with at(t + 12):
                    emit_w_group(b, 3, 0, wT)
                    emit_w_group(b, 3, 1, wT)
                with at(t + 13):
                    for m in range(LT):
                        emit_gemm_full(m, wT, xT, ot)
            else:
                with at(t + 8):
                    for m in range(LT):
                        emit_gemm_half(m, 0, wT, xT, ot)
                with at(t + 10):
                    emit_mul(b, 2)
                    if b < 3:
                        emit_nz_load(b + 1, 2, 4)
                with at(t + 11):
                    emit_w_group(b, 2, 0, wT)
                    emit_w_group(b, 2, 1, wT)
                with at(t + 12):
                    emit_mul(b, 3)
                with at(t + 13):
                    emit_w_group(b, 3, 0, wT)
                    emit_w_group(b, 3, 1, wT)
                if b < 3:
                    with at(t + 14):
                        for m in range(LT):
                            emit_gemm_half(m, 1, wT, xT, ot)
                else:
                    # sample 3 tail: quarter GEMMs so only chunk-3-dependent
                    # work sits behind the final noise bytes
                    with at(t + 12):
                        for m in range(LT):
                            emit_gemm_quarter(m, 2, wT, xT, ot)
                    with at(t + 15):
                        emit_gemm_quarter(0, 3, wT, xT, ot)
                        emit_gemm_quarter(1, 3, wT, xT, ot)
                    with at(t + 17):
                        emit_store(3, ot, 0, 2)
                        emit_gemm_quarter(2, 3, wT, xT, ot)
                        emit_gemm_quarter(3, 3, wT, xT, ot)
                    with at(t + 19):
                        emit_store(3, ot, 2, 4)

        # deferred stores: behind the last noise load in the stream
        with at(88):
            emit_store(0, ots[0])
        with at(90):
            emit_store(1, ots[1])
        with at(92):
            emit_store(2, ots[2])

    _split_multi_waits(nc, mybir)
    return nc


def _get_nc(use_f32r=True):
    key = ("nc", use_f32r)
    if key not in _cache:
        _cache[key] = build_nc(use_f32r)
    return _cache[key]


def kernel(x, weight_mean, weight_logvar, bias, noise):
    from concourse import bass_utils

    x = np.ascontiguousarray(x, dtype=np.float32)
    noise = np.ascontiguousarray(noise, dtype=np.float32)
    weight_mean = np.ascontiguousarray(weight_mean, dtype=np.float32)
    weight_logvar = np.ascontiguousarray(weight_logvar, dtype=np.float32)
    bias = np.ascontiguousarray(bias, dtype=np.float32)

    nc = _get_nc()
    in_maps = []
    for c in range(N_CORES):
        sl = slice(SAMPLES * c, SAMPLES * (c + 1))
        in_maps.append({
            "x": x[sl], "noise": noise[sl],
            "weight_mean": weight_mean, "weight_logvar": weight_logvar,
            "bias": bias,
        })
    res = bass_utils.run_bass_kernel_spmd(nc, in_maps, list(range(N_CORES)))
    out = np.concatenate([np.asarray(res.results[c]["out"]).astype(np.float32)
                          for c in range(N_CORES)], axis=0)
    return out
